# revision 29
# baseline (speedup 1.0000x reference)
"""Capsule-routing kernel v3 — block-diagonal 128x128 stationaries.

Per core: 8 batches in 2 groups of 4 (GB=4), lockstep so vector/scalar ops run
at full [128, *] width and every matmul carries a 128-wide stationary.

Index conventions (per group of GB=4 batches):
  capsule n = 2*kc + par      (kc in [0,16), par in {0,1})
  slot(b, kc) = b*16 + kc     in [0, 64)
  p'(par, slot) = par*64 + slot   -> o / oT / rs2 column order
  b/c layout: [128 (par*64 + m), 32 g, 64 slot]   (m = G in [0,64))

Matmul structure (all stationaries 128 cols -> FWL; one MM per slot):
  zstep: lhsT = wt2[:, g, :] [64 d, 128 (h,i)] (h-dup), rhs = oT [64 d, 128 p']
         -> z_ps[(h,i), g, p']; keep halves: z2v[(h,i), g, slot] = z[h*64+slot][g,i]
  dbstep: lhsT = xtbd[b][:, kc, :] block-diag [[X_A^T,0],[0,X_B^T]] (A=2kc, B=2kc+1),
          rhs = z2v[:, :, slot] -> out rows 0:64 = delta_A[m,g], 64:128 = delta_B.
  pstep:  lhsT = xbd[b][:, kc, :] block-diag over (par,G)x(ch,i),
          rhs = c_sb[:, :, slot] -> rows 0:64 = P_par0[i,g], 64:128 = P_par1.
  mm2:    lhsT = p_allz[:, g, :] [128 (h,i), 128 p'] zero-padded (h==par(p') only),
          rhs = w2[:, g, :] [128 (h,i), 64 d] -> o_ps[p', d] accumulated over g.

Scalar engine uses only the natural_log_exp table set: softmax Exp, squash
sqrt(s) = Exp(0.5*Ln(s)) — no ACT table switches after the first load.
"""

import numpy as np

B, IN_CAPS, IN_DIM = 64, 2048, 64
NUM, DIM = 32, 64
N_CORES = 8
BPC = B // N_CORES  # 8 batches per core
GB = 4              # batches per merged group
NG = BPC // GB      # 2 groups
EPS = 1e-7

_CACHE = {}


def _build_nc(bpc=BPC):
    import concourse.bacc as bacc
    import concourse.tile as tile
    from concourse import mybir

    f32 = mybir.dt.float32
    bf16 = mybir.dt.bfloat16
    Act = mybir.ActivationFunctionType
    Alu = mybir.AluOpType

    ng = bpc // GB
    nc = bacc.Bacc("TRN2", target_bir_lowering=False, debug=False, num_devices=N_CORES)

    # ---- DRAM I/O (per-core shapes) ----
    # xtbd[b][(h,i), kc, (ch,m)] = X[b, (2kc+h)*64+m, i] if ch==h else 0
    xtbd_d = nc.dram_tensor("xtbd", [bpc, 128, 16, 128], bf16, kind="ExternalInput")
    # xbd[b][(par,G), kc, (ch,i)] = X[b, (2kc+par)*64+G, i] if ch==par else 0
    xbd_d = nc.dram_tensor("xbd", [bpc, 128, 16, 128], bf16, kind="ExternalInput")
    # xs[grp, i, p'] = sum_G X[b, n*64+G, i] at p' = par*64 + b*16 + kc
    xs_d = nc.dram_tensor("xs", [ng, IN_DIM, 128], bf16, kind="ExternalInput")
    # w2[(h,i), g, d] = W[i, g*64+d] (h-dup)
    w2_d = nc.dram_tensor("w2", [128, NUM, DIM], bf16, kind="ExternalInput")
    # wt2[d, g, (h,i)] = W[i, g*64+d] (h-dup)
    wt2_d = nc.dram_tensor("wt2", [IN_DIM, 32, 128], bf16, kind="ExternalInput")
    wsum_d = nc.dram_tensor("wsum", [IN_DIM, DIM], bf16, kind="ExternalInput")
    e2_d = nc.dram_tensor("e2", [128, 64], bf16, kind="ExternalInput")
    e3_d = nc.dram_tensor("e3", [64, 128], bf16, kind="ExternalInput")
    out_d = nc.dram_tensor("out", [bpc, NUM, DIM], f32, kind="ExternalOutput")

    with tile.TileContext(nc) as tc:
        with (
            tc.tile_pool(name="const", bufs=1) as cpool,
            tc.tile_pool(name="inp", bufs=2) as ipool,
            tc.tile_pool(name="work", bufs=2) as wpool,
            tc.tile_pool(name="big", bufs=2) as bigpool,
            tc.tile_pool(name="bpool", bufs=4) as bpool,
            tc.tile_pool(name="ps_z", bufs=2, space="PSUM") as ps_z,
            tc.tile_pool(name="ps_bmm", bufs=2, space="PSUM") as ps_bmm,
            tc.tile_pool(name="ps_o", bufs=2, space="PSUM") as ps_o,
        ):
            wsum_t = cpool.tile([IN_DIM, DIM], bf16, tag="wsum")
            nc.sync.dma_start(wsum_t[:], wsum_d[:])
            wt2_t = cpool.tile([IN_DIM, 32, 128], bf16, tag="wt2")
            nc.sync.dma_start(wt2_t[:], wt2_d[:])
            e2_t = cpool.tile([128, 64], bf16, tag="e2")
            nc.sync.dma_start(e2_t[:], e2_d[:])
            e3_t = cpool.tile([64, 128], bf16, tag="e3")
            nc.sync.dma_start(e3_t[:], e3_d[:])
            w2_t = cpool.tile([128, NUM, DIM], bf16, tag="w2")
            eps_t = cpool.tile([128, 1], f32, tag="eps")
            nc.vector.memset(eps_t[:], EPS)
            one_eps_t = cpool.tile([128, 1], f32, tag="oneeps")
            nc.vector.memset(one_eps_t[:], 1.0 + EPS)

            # Preload the one ACT table set holding Exp+Ln+Square+Copy (id 6 =
            # natural_log_exp_and_others) so the compiler inserts no more loads.
            nc.scalar.add_instruction(
                mybir.InstLoadActFuncSet(
                    name=nc.get_next_instruction_name(),
                    act_func_set_id=6, ins=[], outs=[],
                )
            )

            ones64_1 = cpool.tile([IN_DIM, 1], bf16, tag="on641")
            nc.vector.memset(ones64_1[:], 1.0)
            ones1_64 = cpool.tile([1, IN_DIM], bf16, tag="on164")
            nc.vector.memset(ones1_64[:], 1.0)

            # p_allz zero-quads are static: memset once per rotating buffer.
            pz_bufs = []
            for r in range(2):
                t = bigpool.tile([128, 32, 128], bf16, tag="pz")
                pz_bufs.append(t)
            for t in pz_bufs:
                nc.vector.memset(t[0:64, :, 64:128], 0.0)
                nc.scalar.memzero(t[64:128, :, 0:64])

            def squash(o_ps, want_f32=True):
                """psum [128,64] f32 -> f32 sbuf squashed (final output path)."""
                o_sb = wpool.tile([128, DIM], f32, tag="osb")
                nc.vector.tensor_copy(o_sb[:], o_ps[:])
                o2 = wpool.tile([128, DIM], f32, tag="o2")
                s0 = wpool.tile([128, 1], f32, tag="s0")
                nc.scalar.activation(o2[:], o_ps[:], Act.Square, accum_out=s0[:])
                lns = wpool.tile([128, 1], f32, tag="lns")
                nc.scalar.activation(lns[:], s0[:], Act.Ln, bias=eps_t[:])
                u = wpool.tile([128, 1], f32, tag="u")
                nc.scalar.activation(u[:], lns[:], Act.Exp, scale=0.5)
                v = wpool.tile([128, 1], f32, tag="v")
                nc.vector.tensor_scalar_add(v[:], s0[:], 1.0 + EPS)
                rv = wpool.tile([128, 1], f32, tag="rv")
                nc.vector.reciprocal(rv[:], v[:])
                f = wpool.tile([128, 1], f32, tag="f")
                nc.vector.tensor_mul(f[:], u[:], rv[:])
                o_f32 = wpool.tile([128, DIM], f32, tag="osqf")
                nc.vector.tensor_scalar_mul(o_f32[:], o_sb[:], f[:])
                return o_f32, None

            def squash_T(o_psT):
                """psum o^T [64,128] f32 -> oT_sq sbuf [64,128] bf16 squashed.
                s[p'] = sum_d o^2 via ones-fold matmul; f = sqrt(s)/(1+s)."""
                o2T = wpool.tile([IN_DIM, 128], bf16, tag="o2T")
                nc.scalar.activation(o2T[:], o_psT[:], Act.Square)
                s_ps = ps_bmm.tile([1, 128], f32, tag="bmm")
                nc.tensor.matmul(s_ps[:], lhsT=ones64_1[:], rhs=o2T[:], start=True, stop=True)
                ln1 = wpool.tile([1, 128], f32, tag="ln1T")
                nc.scalar.activation(ln1[:], s_ps[:], Act.Ln, bias=eps_t[0:1])
                ln2 = wpool.tile([1, 128], f32, tag="ln2T")
                nc.scalar.activation(ln2[:], s_ps[:], Act.Ln, bias=one_eps_t[0:1])
                df = wpool.tile([1, 128], f32, tag="dfT")
                nc.vector.scalar_tensor_tensor(
                    df[:], ln1[:], 0.5, ln2[:], Alu.mult, Alu.subtract,
                )
                fr = wpool.tile([1, 128], bf16, tag="frT")
                nc.scalar.activation(fr[:], df[:], Act.Exp)
                fm_ps = ps_bmm.tile([IN_DIM, 128], f32, tag="bmm")
                nc.tensor.matmul(fm_ps[:], lhsT=ones1_64[:], rhs=fr[:], start=True, stop=True)
                fmat = wpool.tile([IN_DIM, 128], bf16, tag="fmat")
                nc.vector.tensor_copy(fmat[:], fm_ps[:])
                oT_sq = wpool.tile([IN_DIM, 128], bf16, tag="oTsq")
                nc.vector.tensor_mul(oT_sq[:], o_psT[:], fmat[:])
                return oT_sq

            def zstep(oT):
                """oT [64,128] -> z2v sbuf [128 (h,i), 32 g, 64 slot] bf16
                where half h holds z[p' = h*64 + slot][g, i]."""
                z2v = bigpool.tile([128, 32, 64], bf16, tag="z2v")
                for gw in range(4):  # waves of 8 g
                    z_ps = ps_z.tile([128, 8, 128], f32, tag="z")
                    for j in range(8):
                        g = gw * 8 + j
                        nc.tensor.matmul(
                            z_ps[:, j, :], lhsT=wt2_t[:, g, :], rhs=oT[:],
                            start=True, stop=True,
                        )
                    dst = z2v[:, gw * 8: gw * 8 + 8, :]
                    if gw % 2 == 0:
                        nc.scalar.copy(dst[0:64], z_ps[0:64, :, 0:64])
                        nc.vector.tensor_copy(dst[64:128], z_ps[64:128, :, 64:128])
                    else:
                        nc.vector.tensor_copy(dst[0:64], z_ps[0:64, :, 0:64])
                        nc.scalar.copy(dst[64:128], z_ps[64:128, :, 64:128])
                return z2v

            def dbstep(z2v, xtbd_g, b_prev):
                """-> new b sbuf [128, 32 g, 64 slot] f32."""
                nb = bpool.tile([128, 32, 64], f32, tag="b")
                for bw in range(GB):  # one wave per batch: 16 slots
                    db_ps = ps_bmm.tile([128, 16, 32], f32, tag="bmm")
                    for kc in range(16):
                        slot = bw * 16 + kc
                        nc.tensor.matmul(
                            db_ps[:, kc, :],
                            lhsT=xtbd_g[bw][:, kc, :],
                            rhs=z2v[:, :, slot],
                            start=True, stop=True,
                        )
                    dst = nb[:, :, bw * 16: (bw + 1) * 16]
                    src_db = db_ps[:].rearrange("p kc g -> p g kc")
                    if b_prev is None:
                        if bw % 2 == 0:
                            nc.vector.tensor_copy(dst, src_db)
                        else:
                            nc.scalar.copy(dst, src_db)
                    else:
                        src_prev = b_prev[:, :, bw * 16: (bw + 1) * 16]
                        nc.vector.tensor_add(dst, src_prev, src_db)
                return nb

            def smp_fused(b_sb, xbd_g, p_allz):
                """softmax + pstep pipelined in 2 batch-halves so pstep MMs
                start earlier and the tail chain shortens."""
                expb = bigpool.tile([128, 32, 64], bf16, tag="expb")
                c_sb = bigpool.tile([128, 32, 64], bf16, tag="c")
                for half in range(2):
                    sl = slice(half * 32, half * 32 + 32)
                    nc.scalar.activation(expb[:, :, sl], b_sb[:, :, sl], Act.Exp)
                    Tb = wpool.tile([128, 32, 2], bf16, tag=f"Tb{half}")
                    with nc.allow_low_precision(reason="softmax sums; 2e-2 tol"):
                        nc.vector.tensor_reduce(
                            Tb[:],
                            expb[:, :, sl].rearrange("p g (b kc) -> p g b kc", kc=16),
                            mybir.AxisListType.X, Alu.add,
                        )
                    S_ps = ps_bmm.tile([IN_DIM, 64], f32, tag="bmm")
                    nc.tensor.matmul(
                        S_ps[:], lhsT=e2_t[:],
                        rhs=Tb[:].rearrange("p g b -> p (g b)"),
                        start=True, stop=True,
                    )
                    rs = wpool.tile([IN_DIM, 64], bf16, tag=f"rs{half}")
                    with nc.allow_low_precision(reason="softmax denom; 2e-2 tol"):
                        nc.vector.reciprocal(rs[:], S_ps[:])
                    rs2_ps = ps_bmm.tile([128, 64], f32, tag="bmm")
                    nc.tensor.matmul(rs2_ps[:], lhsT=e3_t[:], rhs=rs[:], start=True, stop=True)
                    rs2 = wpool.tile([128, 32, 2], bf16, tag=f"rs2{half}")
                    nc.vector.tensor_copy(rs2[:].rearrange("p g b -> p (g b)"), rs2_ps[:])
                    nc.vector.tensor_mul(
                        c_sb[:, :, sl].rearrange("p g (b kc) -> p g b kc", kc=16),
                        expb[:, :, sl].rearrange("p g (b kc) -> p g b kc", kc=16),
                        rs2[:, :, :, None].to_broadcast([128, 32, 2, 16]),
                    )
                    for bw in (2 * half, 2 * half + 1):
                        p_ps = ps_bmm.tile([128, 16, 32], f32, tag="bmm")
                        for kc in range(16):
                            slot = bw * 16 + kc
                            nc.tensor.matmul(
                                p_ps[:, kc, :],
                                lhsT=xbd_g[bw][:, kc, :],
                                rhs=c_sb[:, :, slot],
                                start=True, stop=True,
                            )
                        lo = bw * 16
                        src_lo = p_ps[0:64].rearrange("p kc g -> p g kc")
                        src_hi = p_ps[64:128].rearrange("p kc g -> p g kc")
                        if bw % 2 == 0:
                            nc.scalar.copy(p_allz[0:64, :, lo:lo + 16], src_lo)
                            nc.vector.tensor_copy(
                                p_allz[64:128, :, 64 + lo:64 + lo + 16], src_hi)
                        else:
                            nc.vector.tensor_copy(p_allz[0:64, :, lo:lo + 16], src_lo)
                            nc.scalar.copy(
                                p_allz[64:128, :, 64 + lo:64 + lo + 16], src_hi)

            def mm2T(p_allz, o_psT):
                # o^T[d, p'] accumulated over g; stationary = W block (64 cols)
                for g in range(32):
                    nc.tensor.matmul(
                        o_psT[:],
                        lhsT=w2_t[:, g, :],
                        rhs=p_allz[:, g, :],
                        start=(g == 0), stop=(g == 31),
                    )

            def mm2(p_allz, o_ps):
                for g in range(32):
                    nc.tensor.matmul(
                        o_ps[:],
                        lhsT=p_allz[:, g, :],
                        rhs=w2_t[:, g, :],
                        start=(g == 0), stop=(g == 31),
                    )

            # ================= interleaved group emission =================
            st = [dict() for _ in range(ng)]

            pending_bulk = []

            def _load_bulk(grp, s_):
                qs = [nc.sync, nc.gpsimd]
                s_["xtbd_g"], s_["xbd_g"] = [], []
                for j in range(GB):
                    b = grp * GB + j
                    xt = ipool.tile([128, 16, 128], bf16, tag=f"xtbd{j}")
                    qs[j % 2].dma_start(xt[:], xtbd_d[b])
                    s_["xtbd_g"].append(xt)
                for j in range(GB):
                    b = grp * GB + j
                    xt = ipool.tile([128, 16, 128], bf16, tag=f"xbd{j}")
                    qs[(j + 1) % 2].dma_start(xt[:], xbd_d[b])
                    s_["xbd_g"].append(xt)

            def ph_load(g_):
                grp, s_ = g_, st[g_]
                xs_t = ipool.tile([IN_DIM, 128], bf16, tag="xs")
                nc.sync.dma_start(xs_t[:], xs_d[grp])
                s_["xs"] = xs_t
                if grp == 0:
                    _load_bulk(grp, s_)
                    nc.gpsimd.dma_start(w2_t[:], w2_d[:])
                else:
                    # defer group-1 bulk so group-0's loads own the queues
                    pending_bulk.append((grp, s_))

            def ph_flush_loads():
                while pending_bulk:
                    grp, s_ = pending_bulk.pop(0)
                    _load_bulk(grp, s_)

            def ph_iter0(g_):
                s_ = st[g_]
                o_psT = ps_o.tile([IN_DIM, 128], f32, tag="o")
                nc.tensor.matmul(o_psT[:], lhsT=wsum_t[:], rhs=s_["xs"][:], start=True, stop=True)
                s_["oT_sq"] = squash_T(o_psT)
                s_["b"] = None
                s_["m2s_count"] = 0

            def ph_tz(g_):
                s_ = st[g_]
                s_["z2v"] = zstep(s_["oT_sq"])

            def ph_db(g_):
                s_ = st[g_]
                if g_ == 0:
                    ph_flush_loads()
                s_["b"] = dbstep(s_["z2v"], s_["xtbd_g"], s_["b"])

            def ph_smp(g_):
                s_ = st[g_]
                s_["pz"] = pz_bufs[g_ % 2]
                smp_fused(s_["b"], s_["xbd_g"], s_["pz"])

            def ph_m2s(g_):
                s_ = st[g_]
                s_["m2s_count"] += 1
                if s_["m2s_count"] < 2:
                    o_psT = ps_o.tile([IN_DIM, 128], f32, tag="o")
                    mm2T(s_["pz"], o_psT)
                    s_["oT_sq"] = squash_T(o_psT)
                else:
                    o_ps = ps_o.tile([128, DIM], f32, tag="o")
                    mm2(s_["pz"], o_ps)
                    s_["o_f32"], _ = squash(o_ps, want_f32=True)

            def ph_out(g_):
                grp, s_ = g_, st[g_]
                qs = [nc.gpsimd, nc.sync]
                for j in range(GB):
                    b = grp * GB + j
                    for parity in range(2):
                        qs[(2 * j + parity) % 2].dma_start(
                            out_d[b].rearrange("(kc par) d -> par kc d", par=2)[parity],
                            s_["o_f32"][parity * 64 + j * 16: parity * 64 + (j + 1) * 16, :],
                        )

            phases = [ph_load, ph_iter0, ph_tz, ph_db, ph_smp, ph_m2s,
                      ph_tz, ph_db, ph_smp, ph_m2s, ph_out]
            OFFSET = 1
            for k in range(len(phases) + OFFSET * (ng - 1)):
                for grp in range(ng):
                    kk = k - OFFSET * grp
                    if 0 <= kk < len(phases):
                        phases[kk](grp)

    nc.compile()
    return nc


def _get_nc():
    if "nc" not in _CACHE:
        _CACHE["nc"] = _build_nc()
    return _CACHE["nc"]


def _prep_host_small(inputs, kern):
    """Host-side input prep; inputs [Bn, 2048, 64] with Bn a multiple of GB."""
    import ml_dtypes

    bf = ml_dtypes.bfloat16
    Bn = inputs.shape[0]
    ng = Bn // GB
    X = np.ascontiguousarray(inputs, dtype=np.float32)
    W = np.ascontiguousarray(kern.reshape(IN_DIM, NUM * DIM), dtype=np.float32)

    # X blocked: xr[b, kc, par, m, i] = X[b, (2kc+par)*64+m, i]
    xr = X.reshape(Bn, 16, 2, 64, IN_DIM)

    # xtbd[b][(h,i), kc, (ch,m)] = X[b, (2kc+h)*64+m, i] if ch==h else 0
    xtbd = np.zeros((Bn, 2, IN_DIM, 16, 2, 64), np.float32)
    xt_blk = xr.transpose(0, 2, 4, 1, 3)  # [b, par, i, kc, m]
    for h in range(2):
        xtbd[:, h, :, :, h, :] = xt_blk[:, h]
    xtbd_h = xtbd.reshape(Bn, 128, 16, 128)

    # xbd[b][(par,G), kc, (ch,i)] = X[b, (2kc+par)*64+G, i] if ch==par else 0
    xbd = np.zeros((Bn, 2, 64, 16, 2, IN_DIM), np.float32)
    x_blk = xr.transpose(0, 2, 3, 1, 4)  # [b, par, m(G), kc, i]
    for h in range(2):
        xbd[:, h, :, :, h, :] = x_blk[:, h]
    xbd_h = xbd.reshape(Bn, 128, 16, 128)

    # xs[grp, i, par*64 + bj*16 + kc] = sum_G X[b, (2kc+par)*64+G, i]
    xsum = xr.sum(axis=3)  # [b, kc, par, i]
    xs_h = np.zeros((ng, IN_DIM, 128), np.float32)
    for grp in range(ng):
        for j in range(GB):
            for parity in range(2):
                blk = xsum[grp * GB + j, :, parity, :].T  # [i, kc]
                xs_h[grp, :, parity * 64 + j * 16: parity * 64 + (j + 1) * 16] = blk

    Wr = W.reshape(IN_DIM, 32, 64)
    # w2[(h,i), g, d] = W[i, g*64+d]
    w2_h = np.broadcast_to(Wr[None], (2, IN_DIM, 32, 64)).reshape(128, 32, 64)
    w2_h = np.ascontiguousarray(w2_h)
    # wt2[d, g, (h,i)] = W[i, g*64+d]
    wt = Wr.transpose(2, 1, 0)  # [d, g, i]
    wt2_h = np.ascontiguousarray(
        np.broadcast_to(wt[:, :, None, :], (IN_DIM, 32, 2, IN_DIM)).reshape(IN_DIM, 32, 128)
    )
    wsum_h = np.ascontiguousarray(Wr.sum(axis=1) / 32.0)
    eye64 = np.eye(64, dtype=np.float32)
    e2_h = np.concatenate([eye64, eye64], axis=0)  # [128, 64]
    e3_h = np.ascontiguousarray(e2_h.T)            # [64, 128]
    return (
        xtbd_h.astype(bf), xbd_h.astype(bf), xs_h.astype(bf),
        w2_h.astype(bf), wt2_h.astype(bf), wsum_h.astype(bf),
        e2_h.astype(bf), e3_h.astype(bf),
    )


def _make_in_maps(inputs, kern):
    xtbd_h, xbd_h, xs_h, w2_h, wt2_h, wsum_h, e2_h, e3_h = _prep_host_small(
        np.asarray(inputs), np.asarray(kern)
    )
    in_maps = []
    for c in range(N_CORES):
        sl = slice(c * BPC, (c + 1) * BPC)
        gsl = slice(c * NG, (c + 1) * NG)
        in_maps.append(
            {
                "xtbd": xtbd_h[sl], "xbd": xbd_h[sl], "xs": xs_h[gsl],
                "w2": w2_h, "wt2": wt2_h, "wsum": wsum_h,
                "e2": e2_h, "e3": e3_h,
            }
        )
    return in_maps


def kernel(inputs, kernel, num_capsule=NUM, dim_capsule=DIM, routings=3, **_):
    from concourse.bass_utils import run_bass_kernel_spmd

    assert int(num_capsule) == NUM and int(dim_capsule) == DIM and int(routings) == 3
    nc = _get_nc()
    in_maps = _make_in_maps(inputs, kernel)
    res = run_bass_kernel_spmd(nc, in_maps, core_ids=list(range(N_CORES)))
    out = np.concatenate([res.results[c]["out"] for c in range(N_CORES)], axis=0)
    return out.astype(np.float32)


# revision 30
# speedup vs baseline: 1.0733x; 1.0733x over previous
"""Capsule-routing kernel v3 — block-diagonal 128x128 stationaries.

Per core: 8 batches in 2 groups of 4 (GB=4), lockstep so vector/scalar ops run
at full [128, *] width and every matmul carries a 128-wide stationary.

Index conventions (per group of GB=4 batches):
  capsule n = 2*kc + par      (kc in [0,16), par in {0,1})
  slot(b, kc) = b*16 + kc     in [0, 64)
  p'(par, slot) = par*64 + slot   -> o / oT / rs2 column order
  b/c layout: [128 (par*64 + m), 32 g, 64 slot]   (m = G in [0,64))

Matmul structure (all stationaries 128 cols -> FWL; one MM per slot):
  zstep: lhsT = wt2[:, g, :] [64 d, 128 (h,i)] (h-dup), rhs = oT [64 d, 128 p']
         -> z_ps[(h,i), g, p']; keep halves: z2v[(h,i), g, slot] = z[h*64+slot][g,i]
  dbstep: lhsT = xtbd[b][:, kc, :] block-diag [[X_A^T,0],[0,X_B^T]] (A=2kc, B=2kc+1),
          rhs = z2v[:, :, slot] -> out rows 0:64 = delta_A[m,g], 64:128 = delta_B.
  pstep:  lhsT = xbd[b][:, kc, :] block-diag over (par,G)x(ch,i),
          rhs = c_sb[:, :, slot] -> rows 0:64 = P_par0[i,g], 64:128 = P_par1.
  mm2:    lhsT = p_allz[:, g, :] [128 (h,i), 128 p'] zero-padded (h==par(p') only),
          rhs = w2[:, g, :] [128 (h,i), 64 d] -> o_ps[p', d] accumulated over g.

Scalar engine uses only the natural_log_exp table set: softmax Exp, squash
sqrt(s) = Exp(0.5*Ln(s)) — no ACT table switches after the first load.
"""

import numpy as np

B, IN_CAPS, IN_DIM = 64, 2048, 64
NUM, DIM = 32, 64
N_CORES = 8
BPC = B // N_CORES  # 8 batches per core
GB = 4              # batches per merged group
NG = BPC // GB      # 2 groups
EPS = 1e-7

_CACHE = {}


def _build_nc(bpc=BPC):
    import concourse.bacc as bacc
    import concourse.tile as tile
    from concourse import mybir

    f32 = mybir.dt.float32
    bf16 = mybir.dt.bfloat16
    Act = mybir.ActivationFunctionType
    Alu = mybir.AluOpType

    ng = bpc // GB
    nc = bacc.Bacc("TRN2", target_bir_lowering=False, debug=False, num_devices=N_CORES)

    # ---- DRAM I/O (per-core shapes) ----
    # xtbd[b][(h,i), kc, (ch,m)] = X[b, (2kc+h)*64+m, i] if ch==h else 0
    xtbd_d = nc.dram_tensor("xtbd", [bpc, 128, 16, 128], bf16, kind="ExternalInput")
    # xbd[b][(par,G), kc, (ch,i)] = X[b, (2kc+par)*64+G, i] if ch==par else 0
    xbd_d = nc.dram_tensor("xbd", [bpc, 128, 16, 128], bf16, kind="ExternalInput")
    # xs[grp, i, p'] = sum_G X[b, n*64+G, i] at p' = par*64 + b*16 + kc
    xs_d = nc.dram_tensor("xs", [ng, IN_DIM, 128], bf16, kind="ExternalInput")
    # w2[(h,i), g, d] = W[i, g*64+d] (h-dup)
    w2_d = nc.dram_tensor("w2", [128, NUM, DIM], bf16, kind="ExternalInput")
    # wt2[d, g, (h,i)] = W[i, g*64+d] (h-dup)
    wt2_d = nc.dram_tensor("wt2", [IN_DIM, 32, 128], bf16, kind="ExternalInput")
    wsum_d = nc.dram_tensor("wsum", [IN_DIM, DIM], bf16, kind="ExternalInput")
    e2_d = nc.dram_tensor("e2", [128, 64], bf16, kind="ExternalInput")
    e3_d = nc.dram_tensor("e3", [64, 128], bf16, kind="ExternalInput")
    out_d = nc.dram_tensor("out", [bpc, NUM, DIM], f32, kind="ExternalOutput")

    with tile.TileContext(nc) as tc:
        with (
            tc.tile_pool(name="const", bufs=1) as cpool,
            tc.tile_pool(name="inp", bufs=2) as ipool,
            tc.tile_pool(name="work", bufs=2) as wpool,
            tc.tile_pool(name="big", bufs=2) as bigpool,
            tc.tile_pool(name="bpool", bufs=4) as bpool,
            tc.tile_pool(name="ps_z", bufs=2, space="PSUM") as ps_z,
            tc.tile_pool(name="ps_db", bufs=2, space="PSUM") as ps_db,
            tc.tile_pool(name="ps_p", bufs=2, space="PSUM") as ps_p,
            tc.tile_pool(name="ps_o", bufs=2, space="PSUM") as ps_o,
        ):
            wsum_t = cpool.tile([IN_DIM, DIM], bf16, tag="wsum")
            nc.sync.dma_start(wsum_t[:], wsum_d[:])
            wt2_t = cpool.tile([IN_DIM, 32, 128], bf16, tag="wt2")
            nc.sync.dma_start(wt2_t[:], wt2_d[:])
            e2_t = cpool.tile([128, 64], bf16, tag="e2")
            nc.sync.dma_start(e2_t[:], e2_d[:])
            e3_t = cpool.tile([64, 128], bf16, tag="e3")
            nc.sync.dma_start(e3_t[:], e3_d[:])
            w2_t = cpool.tile([128, NUM, DIM], bf16, tag="w2")
            eps_t = cpool.tile([128, 1], f32, tag="eps")
            nc.vector.memset(eps_t[:], EPS)
            one_eps_t = cpool.tile([128, 1], f32, tag="oneeps")
            nc.vector.memset(one_eps_t[:], 1.0 + EPS)

            # Preload the one ACT table set holding Exp+Ln+Square+Copy (id 6 =
            # natural_log_exp_and_others) so the compiler inserts no more loads.
            nc.scalar.add_instruction(
                mybir.InstLoadActFuncSet(
                    name=nc.get_next_instruction_name(),
                    act_func_set_id=6, ins=[], outs=[],
                )
            )

            ones64_1 = cpool.tile([IN_DIM, 1], bf16, tag="on641")
            nc.vector.memset(ones64_1[:], 1.0)
            ones1_64 = cpool.tile([1, IN_DIM], bf16, tag="on164")
            nc.vector.memset(ones1_64[:], 1.0)

            # p_allz zero-quads are static: memset once per rotating buffer.
            pz_bufs = []
            for r in range(2):
                t = bigpool.tile([128, 32, 128], bf16, tag="pz")
                pz_bufs.append(t)
            for t in pz_bufs:
                nc.vector.memset(t[0:64, :, 64:128], 0.0)
                nc.scalar.memzero(t[64:128, :, 0:64])

            def squash(o_ps, want_f32=True):
                """psum [128,64] f32 -> f32 sbuf squashed (final output path)."""
                o_sb = wpool.tile([128, DIM], f32, tag="osb")
                nc.vector.tensor_copy(o_sb[:], o_ps[:])
                o2 = wpool.tile([128, DIM], f32, tag="o2")
                s0 = wpool.tile([128, 1], f32, tag="s0")
                nc.scalar.activation(o2[:], o_ps[:], Act.Square, accum_out=s0[:])
                lns = wpool.tile([128, 1], f32, tag="lns")
                nc.scalar.activation(lns[:], s0[:], Act.Ln, bias=eps_t[:])
                u = wpool.tile([128, 1], f32, tag="u")
                nc.scalar.activation(u[:], lns[:], Act.Exp, scale=0.5)
                v = wpool.tile([128, 1], f32, tag="v")
                nc.vector.tensor_scalar_add(v[:], s0[:], 1.0 + EPS)
                rv = wpool.tile([128, 1], f32, tag="rv")
                nc.vector.reciprocal(rv[:], v[:])
                f = wpool.tile([128, 1], f32, tag="f")
                nc.vector.tensor_mul(f[:], u[:], rv[:])
                o_f32 = wpool.tile([128, DIM], f32, tag="osqf")
                nc.vector.tensor_scalar_mul(o_f32[:], o_sb[:], f[:])
                return o_f32, None

            def squash_T(o_psT):
                """psum o^T [64,128] f32 -> oT_sq sbuf [64,128] bf16 squashed.
                s[p'] = sum_d o^2 via ones-fold matmul; f = sqrt(s)/(1+s)."""
                o2T = wpool.tile([IN_DIM, 128], bf16, tag="o2T")
                nc.scalar.activation(o2T[:], o_psT[:], Act.Square)
                s_ps = ps_db.tile([1, 128], f32, tag="db")
                nc.tensor.matmul(s_ps[:], lhsT=ones64_1[:], rhs=o2T[:], start=True, stop=True)
                ln1 = wpool.tile([1, 128], f32, tag="ln1T")
                nc.scalar.activation(ln1[:], s_ps[:], Act.Ln, bias=eps_t[0:1])
                ln2 = wpool.tile([1, 128], f32, tag="ln2T")
                nc.scalar.activation(ln2[:], s_ps[:], Act.Ln, bias=one_eps_t[0:1])
                df = wpool.tile([1, 128], f32, tag="dfT")
                nc.vector.scalar_tensor_tensor(
                    df[:], ln1[:], 0.5, ln2[:], Alu.mult, Alu.subtract,
                )
                fr = wpool.tile([1, 128], bf16, tag="frT")
                nc.scalar.activation(fr[:], df[:], Act.Exp)
                fm_ps = ps_db.tile([IN_DIM, 128], f32, tag="db")
                nc.tensor.matmul(fm_ps[:], lhsT=ones1_64[:], rhs=fr[:], start=True, stop=True)
                fmat = wpool.tile([IN_DIM, 128], bf16, tag="fmat")
                nc.vector.tensor_copy(fmat[:], fm_ps[:])
                oT_sq = wpool.tile([IN_DIM, 128], bf16, tag="oTsq")
                nc.vector.tensor_mul(oT_sq[:], o_psT[:], fmat[:])
                return oT_sq

            def zstep(oT):
                """oT [64,128] -> z2v sbuf [128 (h,i), 32 g, 64 slot] bf16
                where half h holds z[p' = h*64 + slot][g, i]."""
                z2v = bigpool.tile([128, 32, 64], bf16, tag="z2v")
                for gw in range(8):  # waves of 4 g
                    z_ps = ps_z.tile([128, 4, 128], f32, tag="z")
                    for j in range(4):
                        g = gw * 4 + j
                        nc.tensor.matmul(
                            z_ps[:, j, :], lhsT=wt2_t[:, g, :], rhs=oT[:],
                            start=True, stop=True,
                        )
                    dst = z2v[:, gw * 4: gw * 4 + 4, :]
                    if gw % 2 == 0:
                        nc.scalar.copy(dst[0:64], z_ps[0:64, :, 0:64])
                        nc.vector.tensor_copy(dst[64:128], z_ps[64:128, :, 64:128])
                    else:
                        nc.vector.tensor_copy(dst[0:64], z_ps[0:64, :, 0:64])
                        nc.scalar.copy(dst[64:128], z_ps[64:128, :, 64:128])
                return z2v

            def dbstep(z2v, xtbd_g, b_prev):
                """-> new b sbuf [128, 32 g, 64 slot] f32."""
                nb = bpool.tile([128, 32, 64], f32, tag="b")
                for bw in range(GB):  # one wave per batch: 16 slots
                    db_ps = ps_db.tile([128, 16, 32], f32, tag="db")
                    for kc in range(16):
                        slot = bw * 16 + kc
                        nc.tensor.matmul(
                            db_ps[:, kc, :],
                            lhsT=xtbd_g[bw][:, kc, :],
                            rhs=z2v[:, :, slot],
                            start=True, stop=True,
                        )
                    dst = nb[:, :, bw * 16: (bw + 1) * 16]
                    src_db = db_ps[:].rearrange("p kc g -> p g kc")
                    if b_prev is None:
                        if bw % 2 == 0:
                            nc.vector.tensor_copy(dst, src_db)
                        else:
                            nc.scalar.copy(dst, src_db)
                    else:
                        src_prev = b_prev[:, :, bw * 16: (bw + 1) * 16]
                        nc.vector.tensor_add(dst, src_prev, src_db)
                return nb

            def smp_fused(b_sb, xbd_g, p_allz):
                """softmax + pstep pipelined in 2 batch-halves so pstep MMs
                start earlier and the tail chain shortens."""
                expb = bigpool.tile([128, 32, 64], bf16, tag="expb")
                c_sb = bigpool.tile([128, 32, 64], bf16, tag="c")
                for half in range(2):
                    sl = slice(half * 32, half * 32 + 32)
                    nc.scalar.activation(expb[:, :, sl], b_sb[:, :, sl], Act.Exp)
                    Tb = wpool.tile([128, 32, 2], bf16, tag=f"Tb{half}")
                    with nc.allow_low_precision(reason="softmax sums; 2e-2 tol"):
                        nc.vector.tensor_reduce(
                            Tb[:],
                            expb[:, :, sl].rearrange("p g (b kc) -> p g b kc", kc=16),
                            mybir.AxisListType.X, Alu.add,
                        )
                    S_ps = ps_p.tile([IN_DIM, 64], f32, tag="pw")
                    nc.tensor.matmul(
                        S_ps[:], lhsT=e2_t[:],
                        rhs=Tb[:].rearrange("p g b -> p (g b)"),
                        start=True, stop=True,
                    )
                    rs = wpool.tile([IN_DIM, 64], bf16, tag=f"rs{half}")
                    with nc.allow_low_precision(reason="softmax denom; 2e-2 tol"):
                        nc.vector.reciprocal(rs[:], S_ps[:])
                    rs2_ps = ps_p.tile([128, 64], f32, tag="pw")
                    nc.tensor.matmul(rs2_ps[:], lhsT=e3_t[:], rhs=rs[:], start=True, stop=True)
                    rs2 = wpool.tile([128, 32, 2], bf16, tag=f"rs2{half}")
                    nc.vector.tensor_copy(rs2[:].rearrange("p g b -> p (g b)"), rs2_ps[:])
                    nc.vector.tensor_mul(
                        c_sb[:, :, sl].rearrange("p g (b kc) -> p g b kc", kc=16),
                        expb[:, :, sl].rearrange("p g (b kc) -> p g b kc", kc=16),
                        rs2[:, :, :, None].to_broadcast([128, 32, 2, 16]),
                    )
                    for bw in (2 * half, 2 * half + 1):
                        p_ps = ps_p.tile([128, 16, 32], f32, tag="pw")
                        for kc in range(16):
                            slot = bw * 16 + kc
                            nc.tensor.matmul(
                                p_ps[:, kc, :],
                                lhsT=xbd_g[bw][:, kc, :],
                                rhs=c_sb[:, :, slot],
                                start=True, stop=True,
                            )
                        lo = bw * 16
                        src_lo = p_ps[0:64].rearrange("p kc g -> p g kc")
                        src_hi = p_ps[64:128].rearrange("p kc g -> p g kc")
                        if bw % 2 == 0:
                            nc.scalar.copy(p_allz[0:64, :, lo:lo + 16], src_lo)
                            nc.vector.tensor_copy(
                                p_allz[64:128, :, 64 + lo:64 + lo + 16], src_hi)
                        else:
                            nc.vector.tensor_copy(p_allz[0:64, :, lo:lo + 16], src_lo)
                            nc.scalar.copy(
                                p_allz[64:128, :, 64 + lo:64 + lo + 16], src_hi)

            def mm2T(p_allz, o_psT):
                # o^T[d, p'] accumulated over g; stationary = W block (64 cols)
                for g in range(32):
                    nc.tensor.matmul(
                        o_psT[:],
                        lhsT=w2_t[:, g, :],
                        rhs=p_allz[:, g, :],
                        start=(g == 0), stop=(g == 31),
                    )

            def mm2(p_allz, o_ps):
                for g in range(32):
                    nc.tensor.matmul(
                        o_ps[:],
                        lhsT=p_allz[:, g, :],
                        rhs=w2_t[:, g, :],
                        start=(g == 0), stop=(g == 31),
                    )

            # ================= interleaved group emission =================
            st = [dict() for _ in range(ng)]

            pending_bulk = []

            def _load_bulk(grp, s_):
                qs = [nc.sync, nc.gpsimd]
                s_["xtbd_g"], s_["xbd_g"] = [], []
                for j in range(GB):
                    b = grp * GB + j
                    xt = ipool.tile([128, 16, 128], bf16, tag=f"xtbd{j}")
                    qs[j % 2].dma_start(xt[:], xtbd_d[b])
                    s_["xtbd_g"].append(xt)
                for j in range(GB):
                    b = grp * GB + j
                    xt = ipool.tile([128, 16, 128], bf16, tag=f"xbd{j}")
                    qs[(j + 1) % 2].dma_start(xt[:], xbd_d[b])
                    s_["xbd_g"].append(xt)

            def ph_load(g_):
                grp, s_ = g_, st[g_]
                xs_t = ipool.tile([IN_DIM, 128], bf16, tag="xs")
                nc.sync.dma_start(xs_t[:], xs_d[grp])
                s_["xs"] = xs_t
                if grp == 0:
                    _load_bulk(grp, s_)
                    nc.gpsimd.dma_start(w2_t[:], w2_d[:])
                else:
                    # defer group-1 bulk so group-0's loads own the queues
                    pending_bulk.append((grp, s_))

            def ph_flush_loads():
                while pending_bulk:
                    grp, s_ = pending_bulk.pop(0)
                    _load_bulk(grp, s_)

            def ph_iter0(g_):
                s_ = st[g_]
                o_psT = ps_o.tile([IN_DIM, 128], f32, tag="o")
                nc.tensor.matmul(o_psT[:], lhsT=wsum_t[:], rhs=s_["xs"][:], start=True, stop=True)
                s_["oT_sq"] = squash_T(o_psT)
                s_["b"] = None
                s_["m2s_count"] = 0

            def ph_tz(g_):
                s_ = st[g_]
                s_["z2v"] = zstep(s_["oT_sq"])

            def ph_db(g_):
                s_ = st[g_]
                if g_ == 0:
                    ph_flush_loads()
                s_["b"] = dbstep(s_["z2v"], s_["xtbd_g"], s_["b"])

            def ph_smp(g_):
                s_ = st[g_]
                s_["pz"] = pz_bufs[g_ % 2]
                smp_fused(s_["b"], s_["xbd_g"], s_["pz"])

            def ph_m2s(g_):
                s_ = st[g_]
                s_["m2s_count"] += 1
                if s_["m2s_count"] < 2:
                    o_psT = ps_o.tile([IN_DIM, 128], f32, tag="o")
                    mm2T(s_["pz"], o_psT)
                    s_["oT_sq"] = squash_T(o_psT)
                else:
                    o_ps = ps_o.tile([128, DIM], f32, tag="o")
                    mm2(s_["pz"], o_ps)
                    s_["o_f32"], _ = squash(o_ps, want_f32=True)

            def ph_out(g_):
                grp, s_ = g_, st[g_]
                qs = [nc.gpsimd, nc.sync]
                for j in range(GB):
                    b = grp * GB + j
                    for parity in range(2):
                        qs[(2 * j + parity) % 2].dma_start(
                            out_d[b].rearrange("(kc par) d -> par kc d", par=2)[parity],
                            s_["o_f32"][parity * 64 + j * 16: parity * 64 + (j + 1) * 16, :],
                        )

            phases = [ph_load, ph_iter0, ph_tz, ph_db, ph_smp, ph_m2s,
                      ph_tz, ph_db, ph_smp, ph_m2s, ph_out]
            OFFSET = 1
            for k in range(len(phases) + OFFSET * (ng - 1)):
                for grp in range(ng):
                    kk = k - OFFSET * grp
                    if 0 <= kk < len(phases):
                        phases[kk](grp)

    nc.compile()
    return nc


def _get_nc():
    if "nc" not in _CACHE:
        _CACHE["nc"] = _build_nc()
    return _CACHE["nc"]


def _prep_host_small(inputs, kern):
    """Host-side input prep; inputs [Bn, 2048, 64] with Bn a multiple of GB."""
    import ml_dtypes

    bf = ml_dtypes.bfloat16
    Bn = inputs.shape[0]
    ng = Bn // GB
    X = np.ascontiguousarray(inputs, dtype=np.float32)
    W = np.ascontiguousarray(kern.reshape(IN_DIM, NUM * DIM), dtype=np.float32)

    # X blocked: xr[b, kc, par, m, i] = X[b, (2kc+par)*64+m, i]
    xr = X.reshape(Bn, 16, 2, 64, IN_DIM)

    # xtbd[b][(h,i), kc, (ch,m)] = X[b, (2kc+h)*64+m, i] if ch==h else 0
    xtbd = np.zeros((Bn, 2, IN_DIM, 16, 2, 64), np.float32)
    xt_blk = xr.transpose(0, 2, 4, 1, 3)  # [b, par, i, kc, m]
    for h in range(2):
        xtbd[:, h, :, :, h, :] = xt_blk[:, h]
    xtbd_h = xtbd.reshape(Bn, 128, 16, 128)

    # xbd[b][(par,G), kc, (ch,i)] = X[b, (2kc+par)*64+G, i] if ch==par else 0
    xbd = np.zeros((Bn, 2, 64, 16, 2, IN_DIM), np.float32)
    x_blk = xr.transpose(0, 2, 3, 1, 4)  # [b, par, m(G), kc, i]
    for h in range(2):
        xbd[:, h, :, :, h, :] = x_blk[:, h]
    xbd_h = xbd.reshape(Bn, 128, 16, 128)

    # xs[grp, i, par*64 + bj*16 + kc] = sum_G X[b, (2kc+par)*64+G, i]
    xsum = xr.sum(axis=3)  # [b, kc, par, i]
    xs_h = np.zeros((ng, IN_DIM, 128), np.float32)
    for grp in range(ng):
        for j in range(GB):
            for parity in range(2):
                blk = xsum[grp * GB + j, :, parity, :].T  # [i, kc]
                xs_h[grp, :, parity * 64 + j * 16: parity * 64 + (j + 1) * 16] = blk

    Wr = W.reshape(IN_DIM, 32, 64)
    # w2[(h,i), g, d] = W[i, g*64+d]
    w2_h = np.broadcast_to(Wr[None], (2, IN_DIM, 32, 64)).reshape(128, 32, 64)
    w2_h = np.ascontiguousarray(w2_h)
    # wt2[d, g, (h,i)] = W[i, g*64+d]
    wt = Wr.transpose(2, 1, 0)  # [d, g, i]
    wt2_h = np.ascontiguousarray(
        np.broadcast_to(wt[:, :, None, :], (IN_DIM, 32, 2, IN_DIM)).reshape(IN_DIM, 32, 128)
    )
    wsum_h = np.ascontiguousarray(Wr.sum(axis=1) / 32.0)
    eye64 = np.eye(64, dtype=np.float32)
    e2_h = np.concatenate([eye64, eye64], axis=0)  # [128, 64]
    e3_h = np.ascontiguousarray(e2_h.T)            # [64, 128]
    return (
        xtbd_h.astype(bf), xbd_h.astype(bf), xs_h.astype(bf),
        w2_h.astype(bf), wt2_h.astype(bf), wsum_h.astype(bf),
        e2_h.astype(bf), e3_h.astype(bf),
    )


def _make_in_maps(inputs, kern):
    xtbd_h, xbd_h, xs_h, w2_h, wt2_h, wsum_h, e2_h, e3_h = _prep_host_small(
        np.asarray(inputs), np.asarray(kern)
    )
    in_maps = []
    for c in range(N_CORES):
        sl = slice(c * BPC, (c + 1) * BPC)
        gsl = slice(c * NG, (c + 1) * NG)
        in_maps.append(
            {
                "xtbd": xtbd_h[sl], "xbd": xbd_h[sl], "xs": xs_h[gsl],
                "w2": w2_h, "wt2": wt2_h, "wsum": wsum_h,
                "e2": e2_h, "e3": e3_h,
            }
        )
    return in_maps


def kernel(inputs, kernel, num_capsule=NUM, dim_capsule=DIM, routings=3, **_):
    from concourse.bass_utils import run_bass_kernel_spmd

    assert int(num_capsule) == NUM and int(dim_capsule) == DIM and int(routings) == 3
    nc = _get_nc()
    in_maps = _make_in_maps(inputs, kernel)
    res = run_bass_kernel_spmd(nc, in_maps, core_ids=list(range(N_CORES)))
    out = np.concatenate([res.results[c]["out"] for c in range(N_CORES)], axis=0)
    return out.astype(np.float32)


# revision 33
# speedup vs baseline: 1.0913x; 1.0167x over previous
"""Capsule-routing kernel v3 — block-diagonal 128x128 stationaries.

Per core: 8 batches in 2 groups of 4 (GB=4), lockstep so vector/scalar ops run
at full [128, *] width and every matmul carries a 128-wide stationary.

Index conventions (per group of GB=4 batches):
  capsule n = 2*kc + par      (kc in [0,16), par in {0,1})
  slot(b, kc) = b*16 + kc     in [0, 64)
  p'(par, slot) = par*64 + slot   -> o / oT / rs2 column order
  b/c layout: [128 (par*64 + m), 32 g, 64 slot]   (m = G in [0,64))

Matmul structure (all stationaries 128 cols -> FWL; one MM per slot):
  zstep: lhsT = wt2[:, g, :] [64 d, 128 (h,i)] (h-dup), rhs = oT [64 d, 128 p']
         -> z_ps[(h,i), g, p']; keep halves: z2v[(h,i), g, slot] = z[h*64+slot][g,i]
  dbstep: lhsT = xtbd[b][:, kc, :] block-diag [[X_A^T,0],[0,X_B^T]] (A=2kc, B=2kc+1),
          rhs = z2v[:, :, slot] -> out rows 0:64 = delta_A[m,g], 64:128 = delta_B.
  pstep:  lhsT = xbd[b][:, kc, :] block-diag over (par,G)x(ch,i),
          rhs = c_sb[:, :, slot] -> rows 0:64 = P_par0[i,g], 64:128 = P_par1.
  mm2:    lhsT = p_allz[:, g, :] [128 (h,i), 128 p'] zero-padded (h==par(p') only),
          rhs = w2[:, g, :] [128 (h,i), 64 d] -> o_ps[p', d] accumulated over g.

Scalar engine uses only the natural_log_exp table set: softmax Exp, squash
sqrt(s) = Exp(0.5*Ln(s)) — no ACT table switches after the first load.
"""

import numpy as np

B, IN_CAPS, IN_DIM = 64, 2048, 64
NUM, DIM = 32, 64
N_CORES = 8
BPC = B // N_CORES  # 8 batches per core
GB = 4              # batches per merged group
NG = BPC // GB      # 2 groups
EPS = 1e-7

_CACHE = {}


def _build_nc(bpc=BPC):
    import concourse.bacc as bacc
    import concourse.tile as tile
    from concourse import mybir

    f32 = mybir.dt.float32
    bf16 = mybir.dt.bfloat16
    fp8 = mybir.dt.float8e4
    Act = mybir.ActivationFunctionType
    Alu = mybir.AluOpType

    ng = bpc // GB
    nc = bacc.Bacc("TRN2", target_bir_lowering=False, debug=False, num_devices=N_CORES)

    # ---- DRAM I/O (per-core shapes) ----
    # xtbd[b][(h,i), kc, (ch,m)] = X[b, (2kc+h)*64+m, i] if ch==h else 0
    xtbd_d = nc.dram_tensor("xtbd", [bpc, 128, 16, 128], fp8, kind="ExternalInput")
    # xbd[b][(par,G), kc, (ch,i)] = X[b, (2kc+par)*64+G, i] if ch==par else 0
    xbd_d = nc.dram_tensor("xbd", [bpc, 128, 16, 128], bf16, kind="ExternalInput")
    # xs[grp, i, p'] = sum_G X[b, n*64+G, i] at p' = par*64 + b*16 + kc
    xs_d = nc.dram_tensor("xs", [ng, IN_DIM, 128], bf16, kind="ExternalInput")
    # w2[(h,i), g, d] = W[i, g*64+d] (h-dup)
    w2_d = nc.dram_tensor("w2", [128, NUM, DIM], bf16, kind="ExternalInput")
    # wt2[d, g, (h,i)] = W[i, g*64+d] (h-dup)
    wt2_d = nc.dram_tensor("wt2", [IN_DIM, 32, 128], bf16, kind="ExternalInput")
    wsum_d = nc.dram_tensor("wsum", [IN_DIM, DIM], bf16, kind="ExternalInput")
    e2_d = nc.dram_tensor("e2", [128, 64], bf16, kind="ExternalInput")
    e3_d = nc.dram_tensor("e3", [64, 128], bf16, kind="ExternalInput")
    out_d = nc.dram_tensor("out", [bpc, NUM, DIM], f32, kind="ExternalOutput")

    with tile.TileContext(nc) as tc:
        with (
            tc.tile_pool(name="const", bufs=1) as cpool,
            tc.tile_pool(name="inp", bufs=2) as ipool,
            tc.tile_pool(name="work", bufs=2) as wpool,
            tc.tile_pool(name="big", bufs=2) as bigpool,
            tc.tile_pool(name="bpool", bufs=4) as bpool,
            tc.tile_pool(name="ps_z", bufs=2, space="PSUM") as ps_z,
            tc.tile_pool(name="ps_db", bufs=2, space="PSUM") as ps_db,
            tc.tile_pool(name="ps_p", bufs=2, space="PSUM") as ps_p,
            tc.tile_pool(name="ps_o", bufs=2, space="PSUM") as ps_o,
        ):
            wsum_t = cpool.tile([IN_DIM, DIM], bf16, tag="wsum")
            nc.sync.dma_start(wsum_t[:], wsum_d[:])
            wt2_t = cpool.tile([IN_DIM, 32, 128], bf16, tag="wt2")
            nc.sync.dma_start(wt2_t[:], wt2_d[:])
            e2_t = cpool.tile([128, 64], bf16, tag="e2")
            nc.sync.dma_start(e2_t[:], e2_d[:])
            e3_t = cpool.tile([64, 128], bf16, tag="e3")
            nc.sync.dma_start(e3_t[:], e3_d[:])
            w2_t = cpool.tile([128, NUM, DIM], bf16, tag="w2")
            eps_t = cpool.tile([128, 1], f32, tag="eps")
            nc.vector.memset(eps_t[:], EPS)
            one_eps_t = cpool.tile([128, 1], f32, tag="oneeps")
            nc.vector.memset(one_eps_t[:], 1.0 + EPS)

            # Preload the one ACT table set holding Exp+Ln+Square+Copy (id 6 =
            # natural_log_exp_and_others) so the compiler inserts no more loads.
            nc.scalar.add_instruction(
                mybir.InstLoadActFuncSet(
                    name=nc.get_next_instruction_name(),
                    act_func_set_id=6, ins=[], outs=[],
                )
            )

            ones64_1 = cpool.tile([IN_DIM, 1], bf16, tag="on641")
            nc.vector.memset(ones64_1[:], 1.0)
            ones1_64 = cpool.tile([1, IN_DIM], bf16, tag="on164")
            nc.vector.memset(ones1_64[:], 1.0)

            # p_allz zero-quads are static: memset once per rotating buffer.
            pz_bufs = []
            for r in range(2):
                t = bigpool.tile([128, 32, 128], bf16, tag="pz")
                pz_bufs.append(t)
            for t in pz_bufs:
                nc.vector.memset(t[0:64, :, 64:128], 0.0)
                nc.scalar.memzero(t[64:128, :, 0:64])

            def squash(o_ps, want_f32=True):
                """psum [128,64] f32 -> f32 sbuf squashed (final output path)."""
                o_sb = wpool.tile([128, DIM], f32, tag="osb")
                nc.vector.tensor_copy(o_sb[:], o_ps[:])
                o2 = wpool.tile([128, DIM], f32, tag="o2")
                s0 = wpool.tile([128, 1], f32, tag="s0")
                nc.scalar.activation(o2[:], o_ps[:], Act.Square, accum_out=s0[:])
                lns = wpool.tile([128, 1], f32, tag="lns")
                nc.scalar.activation(lns[:], s0[:], Act.Ln, bias=eps_t[:])
                u = wpool.tile([128, 1], f32, tag="u")
                nc.scalar.activation(u[:], lns[:], Act.Exp, scale=0.5)
                v = wpool.tile([128, 1], f32, tag="v")
                nc.vector.tensor_scalar_add(v[:], s0[:], 1.0 + EPS)
                rv = wpool.tile([128, 1], f32, tag="rv")
                nc.vector.reciprocal(rv[:], v[:])
                f = wpool.tile([128, 1], f32, tag="f")
                nc.vector.tensor_mul(f[:], u[:], rv[:])
                o_f32 = wpool.tile([128, DIM], f32, tag="osqf")
                nc.vector.tensor_scalar_mul(o_f32[:], o_sb[:], f[:])
                return o_f32, None

            def squash_T(o_psT):
                """psum o^T [64,128] f32 -> oT_sq sbuf [64,128] bf16 squashed.
                s[p'] = sum_d o^2 via ones-fold matmul; f = sqrt(s)/(1+s)."""
                o2T = wpool.tile([IN_DIM, 128], bf16, tag="o2T")
                nc.scalar.activation(o2T[:], o_psT[:], Act.Square)
                s_ps = ps_db.tile([1, 128], f32, tag="db")
                nc.tensor.matmul(s_ps[:], lhsT=ones64_1[:], rhs=o2T[:], start=True, stop=True)
                ln1 = wpool.tile([1, 128], f32, tag="ln1T")
                nc.scalar.activation(ln1[:], s_ps[:], Act.Ln, bias=eps_t[0:1])
                ln2 = wpool.tile([1, 128], f32, tag="ln2T")
                nc.scalar.activation(ln2[:], s_ps[:], Act.Ln, bias=one_eps_t[0:1])
                df = wpool.tile([1, 128], f32, tag="dfT")
                nc.vector.scalar_tensor_tensor(
                    df[:], ln1[:], 0.5, ln2[:], Alu.mult, Alu.subtract,
                )
                fr = wpool.tile([1, 128], bf16, tag="frT")
                nc.scalar.activation(fr[:], df[:], Act.Exp)
                fm_ps = ps_db.tile([IN_DIM, 128], f32, tag="db")
                nc.tensor.matmul(fm_ps[:], lhsT=ones1_64[:], rhs=fr[:], start=True, stop=True)
                fmat = wpool.tile([IN_DIM, 128], bf16, tag="fmat")
                nc.vector.tensor_copy(fmat[:], fm_ps[:])
                oT_sq = wpool.tile([IN_DIM, 128], bf16, tag="oTsq")
                nc.vector.tensor_mul(oT_sq[:], o_psT[:], fmat[:])
                return oT_sq

            def zstep(oT):
                """oT [64,128] -> z2v sbuf [128 (h,i), 32 g, 64 slot] bf16
                where half h holds z[p' = h*64 + slot][g, i]."""
                z2v = bigpool.tile([128, 32, 64], bf16, tag="z2v")
                for gw in range(8):  # waves of 4 g
                    z_ps = ps_z.tile([128, 4, 128], f32, tag="z")
                    for j in range(4):
                        g = gw * 4 + j
                        nc.tensor.matmul(
                            z_ps[:, j, :], lhsT=wt2_t[:, g, :], rhs=oT[:],
                            start=True, stop=True,
                        )
                    dst = z2v[:, gw * 4: gw * 4 + 4, :]
                    if gw % 2 == 0:
                        nc.scalar.copy(dst[0:64], z_ps[0:64, :, 0:64])
                        nc.vector.tensor_copy(dst[64:128], z_ps[64:128, :, 64:128])
                    else:
                        nc.vector.tensor_copy(dst[0:64], z_ps[0:64, :, 0:64])
                        nc.scalar.copy(dst[64:128], z_ps[64:128, :, 64:128])
                return z2v

            def dbstep(z2v, xtbd_g, b_prev):
                """-> new b sbuf [128, 32 g, 64 slot] f32."""
                nb = bpool.tile([128, 32, 64], f32, tag="b")
                for bw in range(GB):  # one wave per batch: 16 slots
                    db_ps = ps_db.tile([128, 16, 32], f32, tag="db")
                    for kc in range(16):
                        slot = bw * 16 + kc
                        nc.tensor.matmul(
                            db_ps[:, kc, :],
                            lhsT=xtbd_g[bw][:, kc, :],
                            rhs=z2v[:, :, slot],
                            start=True, stop=True,
                        )
                    dst = nb[:, :, bw * 16: (bw + 1) * 16]
                    src_db = db_ps[:].rearrange("p kc g -> p g kc")
                    if b_prev is None:
                        if bw % 2 == 0:
                            nc.vector.tensor_copy(dst, src_db)
                        else:
                            nc.scalar.copy(dst, src_db)
                    else:
                        src_prev = b_prev[:, :, bw * 16: (bw + 1) * 16]
                        nc.vector.tensor_add(dst, src_prev, src_db)
                return nb

            def smp_fused(b_sb, xbd_g, p_allz):
                """softmax + pstep pipelined in 2 batch-halves so pstep MMs
                start earlier and the tail chain shortens."""
                expb = bigpool.tile([128, 32, 64], bf16, tag="expb")
                c_sb = bigpool.tile([128, 32, 64], bf16, tag="c")
                for half in range(2):
                    sl = slice(half * 32, half * 32 + 32)
                    nc.scalar.activation(expb[:, :, sl], b_sb[:, :, sl], Act.Exp)
                    Tb = wpool.tile([128, 32, 2], bf16, tag=f"Tb{half}")
                    with nc.allow_low_precision(reason="softmax sums; 2e-2 tol"):
                        nc.vector.tensor_reduce(
                            Tb[:],
                            expb[:, :, sl].rearrange("p g (b kc) -> p g b kc", kc=16),
                            mybir.AxisListType.X, Alu.add,
                        )
                    S_ps = ps_p.tile([IN_DIM, 64], f32, tag="pw")
                    nc.tensor.matmul(
                        S_ps[:], lhsT=e2_t[:],
                        rhs=Tb[:].rearrange("p g b -> p (g b)"),
                        start=True, stop=True,
                    )
                    rs = wpool.tile([IN_DIM, 64], bf16, tag=f"rs{half}")
                    with nc.allow_low_precision(reason="softmax denom; 2e-2 tol"):
                        nc.vector.reciprocal(rs[:], S_ps[:])
                    rs2_ps = ps_p.tile([128, 64], f32, tag="pw")
                    nc.tensor.matmul(rs2_ps[:], lhsT=e3_t[:], rhs=rs[:], start=True, stop=True)
                    rs2 = wpool.tile([128, 32, 2], bf16, tag=f"rs2{half}")
                    nc.vector.tensor_copy(rs2[:].rearrange("p g b -> p (g b)"), rs2_ps[:])
                    nc.vector.tensor_mul(
                        c_sb[:, :, sl].rearrange("p g (b kc) -> p g b kc", kc=16),
                        expb[:, :, sl].rearrange("p g (b kc) -> p g b kc", kc=16),
                        rs2[:, :, :, None].to_broadcast([128, 32, 2, 16]),
                    )
                    for bw in (2 * half, 2 * half + 1):
                        p_ps = ps_p.tile([128, 16, 32], f32, tag="pw")
                        for kc in range(16):
                            slot = bw * 16 + kc
                            nc.tensor.matmul(
                                p_ps[:, kc, :],
                                lhsT=xbd_g[bw][:, kc, :],
                                rhs=c_sb[:, :, slot],
                                start=True, stop=True,
                            )
                        lo = bw * 16
                        src_lo = p_ps[0:64].rearrange("p kc g -> p g kc")
                        src_hi = p_ps[64:128].rearrange("p kc g -> p g kc")
                        if bw % 2 == 0:
                            nc.scalar.copy(p_allz[0:64, :, lo:lo + 16], src_lo)
                            nc.vector.tensor_copy(
                                p_allz[64:128, :, 64 + lo:64 + lo + 16], src_hi)
                        else:
                            nc.vector.tensor_copy(p_allz[0:64, :, lo:lo + 16], src_lo)
                            nc.scalar.copy(
                                p_allz[64:128, :, 64 + lo:64 + lo + 16], src_hi)

            def mm2T(p_allz, o_psT):
                # o^T[d, p'] accumulated over g; stationary = W block (64 cols)
                for g in range(32):
                    nc.tensor.matmul(
                        o_psT[:],
                        lhsT=w2_t[:, g, :],
                        rhs=p_allz[:, g, :],
                        start=(g == 0), stop=(g == 31),
                    )

            def mm2(p_allz, o_ps):
                for g in range(32):
                    nc.tensor.matmul(
                        o_ps[:],
                        lhsT=p_allz[:, g, :],
                        rhs=w2_t[:, g, :],
                        start=(g == 0), stop=(g == 31),
                    )

            # ================= interleaved group emission =================
            st = [dict() for _ in range(ng)]

            pending_bulk = []

            def _load_bulk(grp, s_):
                qs = [nc.sync, nc.gpsimd]
                s_["xtbd_g"], s_["xbd_g"] = [], []
                for j in range(GB):
                    b = grp * GB + j
                    xt = ipool.tile([128, 16, 128], fp8, tag=f"xtbd{j}")
                    qs[j % 2].dma_start(xt[:], xtbd_d[b])
                    s_["xtbd_g"].append(xt)
                for j in range(GB):
                    b = grp * GB + j
                    xt = ipool.tile([128, 16, 128], bf16, tag=f"xbd{j}")
                    qs[(j + 1) % 2].dma_start(xt[:], xbd_d[b])
                    s_["xbd_g"].append(xt)

            def ph_load(g_):
                grp, s_ = g_, st[g_]
                xs_t = ipool.tile([IN_DIM, 128], bf16, tag="xs")
                nc.sync.dma_start(xs_t[:], xs_d[grp])
                s_["xs"] = xs_t
                if grp == 0:
                    _load_bulk(grp, s_)
                    nc.gpsimd.dma_start(w2_t[:], w2_d[:])
                else:
                    # defer group-1 bulk so group-0's loads own the queues
                    pending_bulk.append((grp, s_))

            def ph_flush_loads():
                while pending_bulk:
                    grp, s_ = pending_bulk.pop(0)
                    _load_bulk(grp, s_)

            def ph_iter0(g_):
                s_ = st[g_]
                o_psT = ps_o.tile([IN_DIM, 128], f32, tag="o")
                nc.tensor.matmul(o_psT[:], lhsT=wsum_t[:], rhs=s_["xs"][:], start=True, stop=True)
                s_["oT_sq"] = squash_T(o_psT)
                s_["b"] = None
                s_["m2s_count"] = 0

            def ph_tz(g_):
                s_ = st[g_]
                s_["z2v"] = zstep(s_["oT_sq"])

            def ph_db(g_):
                s_ = st[g_]
                if g_ == 0:
                    ph_flush_loads()
                s_["b"] = dbstep(s_["z2v"], s_["xtbd_g"], s_["b"])

            def ph_smp(g_):
                s_ = st[g_]
                s_["pz"] = pz_bufs[g_ % 2]
                smp_fused(s_["b"], s_["xbd_g"], s_["pz"])

            def ph_m2s(g_):
                s_ = st[g_]
                s_["m2s_count"] += 1
                if s_["m2s_count"] < 2:
                    o_psT = ps_o.tile([IN_DIM, 128], f32, tag="o")
                    mm2T(s_["pz"], o_psT)
                    s_["oT_sq"] = squash_T(o_psT)
                else:
                    o_ps = ps_o.tile([128, DIM], f32, tag="o")
                    mm2(s_["pz"], o_ps)
                    s_["o_f32"], _ = squash(o_ps, want_f32=True)

            def ph_out(g_):
                grp, s_ = g_, st[g_]
                qs = [nc.gpsimd, nc.sync]
                for j in range(GB):
                    b = grp * GB + j
                    for parity in range(2):
                        qs[(2 * j + parity) % 2].dma_start(
                            out_d[b].rearrange("(kc par) d -> par kc d", par=2)[parity],
                            s_["o_f32"][parity * 64 + j * 16: parity * 64 + (j + 1) * 16, :],
                        )

            phases = [ph_load, ph_iter0, ph_tz, ph_db, ph_smp, ph_m2s,
                      ph_tz, ph_db, ph_smp, ph_m2s, ph_out]
            OFFSET = 1
            for k in range(len(phases) + OFFSET * (ng - 1)):
                for grp in range(ng):
                    kk = k - OFFSET * grp
                    if 0 <= kk < len(phases):
                        phases[kk](grp)

    nc.compile()
    return nc


def _get_nc():
    if "nc" not in _CACHE:
        _CACHE["nc"] = _build_nc()
    return _CACHE["nc"]


def _prep_host_small(inputs, kern):
    """Host-side input prep; inputs [Bn, 2048, 64] with Bn a multiple of GB."""
    import ml_dtypes

    bf = ml_dtypes.bfloat16
    Bn = inputs.shape[0]
    ng = Bn // GB
    X = np.ascontiguousarray(inputs, dtype=np.float32)
    W = np.ascontiguousarray(kern.reshape(IN_DIM, NUM * DIM), dtype=np.float32)

    # X blocked: xr[b, kc, par, m, i] = X[b, (2kc+par)*64+m, i]
    xr = X.reshape(Bn, 16, 2, 64, IN_DIM)

    # xtbd[b][(h,i), kc, (ch,m)] = X[b, (2kc+h)*64+m, i] if ch==h else 0
    xtbd = np.zeros((Bn, 2, IN_DIM, 16, 2, 64), np.float32)
    xt_blk = xr.transpose(0, 2, 4, 1, 3)  # [b, par, i, kc, m]
    for h in range(2):
        xtbd[:, h, :, :, h, :] = xt_blk[:, h]
    xtbd_h = xtbd.reshape(Bn, 128, 16, 128)

    # xbd[b][(par,G), kc, (ch,i)] = X[b, (2kc+par)*64+G, i] if ch==par else 0
    xbd = np.zeros((Bn, 2, 64, 16, 2, IN_DIM), np.float32)
    x_blk = xr.transpose(0, 2, 3, 1, 4)  # [b, par, m(G), kc, i]
    for h in range(2):
        xbd[:, h, :, :, h, :] = x_blk[:, h]
    xbd_h = xbd.reshape(Bn, 128, 16, 128)

    # xs[grp, i, par*64 + bj*16 + kc] = sum_G X[b, (2kc+par)*64+G, i]
    xsum = xr.sum(axis=3)  # [b, kc, par, i]
    xs_h = np.zeros((ng, IN_DIM, 128), np.float32)
    for grp in range(ng):
        for j in range(GB):
            for parity in range(2):
                blk = xsum[grp * GB + j, :, parity, :].T  # [i, kc]
                xs_h[grp, :, parity * 64 + j * 16: parity * 64 + (j + 1) * 16] = blk

    Wr = W.reshape(IN_DIM, 32, 64)
    # w2[(h,i), g, d] = W[i, g*64+d]
    w2_h = np.broadcast_to(Wr[None], (2, IN_DIM, 32, 64)).reshape(128, 32, 64)
    w2_h = np.ascontiguousarray(w2_h)
    # wt2[d, g, (h,i)] = W[i, g*64+d]
    wt = Wr.transpose(2, 1, 0)  # [d, g, i]
    wt2_h = np.ascontiguousarray(
        np.broadcast_to(wt[:, :, None, :], (IN_DIM, 32, 2, IN_DIM)).reshape(IN_DIM, 32, 128)
    )
    wsum_h = np.ascontiguousarray(Wr.sum(axis=1) / 32.0)
    eye64 = np.eye(64, dtype=np.float32)
    e2_h = np.concatenate([eye64, eye64], axis=0)  # [128, 64]
    e3_h = np.ascontiguousarray(e2_h.T)            # [64, 128]
    f8 = ml_dtypes.float8_e4m3fn
    return (
        xtbd_h.astype(f8), xbd_h.astype(bf), xs_h.astype(bf),
        w2_h.astype(bf), wt2_h.astype(bf), wsum_h.astype(bf),
        e2_h.astype(bf), e3_h.astype(bf),
    )


def _make_in_maps(inputs, kern):
    xtbd_h, xbd_h, xs_h, w2_h, wt2_h, wsum_h, e2_h, e3_h = _prep_host_small(
        np.asarray(inputs), np.asarray(kern)
    )
    in_maps = []
    for c in range(N_CORES):
        sl = slice(c * BPC, (c + 1) * BPC)
        gsl = slice(c * NG, (c + 1) * NG)
        in_maps.append(
            {
                "xtbd": xtbd_h[sl], "xbd": xbd_h[sl], "xs": xs_h[gsl],
                "w2": w2_h, "wt2": wt2_h, "wsum": wsum_h,
                "e2": e2_h, "e3": e3_h,
            }
        )
    return in_maps


def kernel(inputs, kernel, num_capsule=NUM, dim_capsule=DIM, routings=3, **_):
    from concourse.bass_utils import run_bass_kernel_spmd

    assert int(num_capsule) == NUM and int(dim_capsule) == DIM and int(routings) == 3
    nc = _get_nc()
    in_maps = _make_in_maps(inputs, kernel)
    res = run_bass_kernel_spmd(nc, in_maps, core_ids=list(range(N_CORES)))
    out = np.concatenate([res.results[c]["out"] for c in range(N_CORES)], axis=0)
    return out.astype(np.float32)
